# revision 1
# baseline (speedup 1.0000x reference)
"""MoE (top-2 of 8 experts, SwiGLU) Trainium2 kernel, expert-parallel over 8 cores.

Contract: kernel(**inputs) takes the FULL unsharded inputs
  x [2,2048,1024] f32, gate_w [8,1024] f32,
  w1 [8,2048,1024] f32, w2 [8,1024,2048] f32, w3 [8,2048,1024] f32
and returns the FULL output [2,2048,1024] f32.

Strategy (expert-parallel, per the hint "replicate the gate and all-to-all the
token dispatch"): routing (gate softmax + top-2) is computed on host; tokens
are dispatched (gathered) per expert; core e runs the SwiGLU FFN of expert e
over its ~N*TOPK/E assigned tokens (padded to capacity C), pre-scaled by the
combine weight; the host scatter-adds the two expert contributions per token.

Device kernel (per core, feature-major layout so no on-device transposes;
fp32r matmuls = full PE rate at moving-dim >= 256, ~1.5e-4 matmul rel err):
  h1T = w1 @ xg^T   [H, C]   (lhsT = w1T block, rhs = xgT)
  h3T = w3 @ xg^T   [H, C]
  aT  = silu(h1T) * h3T      (ACT Silu + DVE mul, PSUM->SBUF)
  yT  = (w2 @ aT) * combine  [D, C]  (DVE mul on PSUM eviction)

Tokens are processed in free-dim chunks of 256..512 (PSUM-bank bound is 512
fp32; fp32r drops to 1/4 rate below 256), sized so the padded capacity C
hugs the max per-expert token count.
"""

import math
import sys

import numpy as np

for _p in ("/opt/trn_rl_repo", "/opt/pypackages"):
    if _p not in sys.path:
        sys.path.append(_p)

import concourse.bass as bass  # noqa: E402
import concourse.tile as tile  # noqa: E402
from concourse import bacc, mybir  # noqa: E402
from concourse.bass_utils import run_bass_kernel_spmd  # noqa: E402

B, T, D, H, E, TOPK = 2, 2048, 1024, 2048, 8, 2
N = B * T
P = 128
KD = D // P   # 8  k-tiles over D
KH = H // P   # 16 k-tiles over H
HB = H // P   # 16 h blocks of 128 (M dim, stage A)
DB = D // P   # 8  d blocks of 128 (M dim, stage B)

F32 = mybir.dt.float32
F32R = mybir.dt.float32r

# set by test.py to capture an NTFF profile; kernel() stores results here
TRACE = False
TRACE_ALL_CORES = False
LAST_RESULTS = None

_program_cache = {}

# CoreSim doesn't implement Silu; simcheck.py overrides this to Sigmoid.
_ACT_FUNC = mybir.ActivationFunctionType.Silu


# Max tokens per expert handled on host when the count barely exceeds a
# 512 multiple (capacity-factor overflow): full 512-wide chunks minimize the
# per-matmul dispatch overhead (768 vs 1152 MMs for cmax ~1071).
OVERFLOW_MAX = 64


def _chunk_plan(cmax: int) -> list[int]:
    """Token-chunk sizes for the device capacity: each <=512 (PSUM bank),
    as equal as possible (keeps every chunk >=256 for full-rate fp32r when
    cmax allows), 32-aligned, minimal total padding. If cmax is within
    OVERFLOW_MAX above a 512 multiple, use full 512 chunks and let the
    caller route the overflow tokens to the host FFN."""
    if cmax >= 512 and cmax - (cmax // 512) * 512 <= OVERFLOW_MAX:
        return [512] * (cmax // 512)
    n = max(1, math.ceil(cmax / 512))
    chunks = []
    rem = cmax
    for i in range(n):
        s = math.ceil(rem / (n - i) / 32) * 32
        s = min(max(s, 256), 512)
        chunks.append(s)
        rem -= s
    return chunks


def _host_ffn(x_rows, w1e, w2e, w3e, wts):
    """Exact host-side SwiGLU FFN for capacity-overflow tokens (<=64/expert)."""
    h1 = x_rows @ w1e.T
    h3 = x_rows @ w3e.T
    a = h1 / (1.0 + np.exp(-h1)) * h3
    return (a @ w2e.T) * wts[:, None]


def _build_program(chunks: list[int]):
    """Bass program for one core: expert FFN over C = sum(chunks) tokens."""
    C = sum(chunks)
    offs = [sum(chunks[:i]) for i in range(len(chunks))]
    tsls = [bass.ds(o, s) for o, s in zip(offs, chunks)]
    nt = len(chunks)

    nc = bacc.Bacc(
        "TRN2", target_bir_lowering=False, debug=False,
        enable_asserts=False, num_devices=8,
    )
    xgT_d = nc.dram_tensor("xgT", [D, C], F32R, kind="ExternalInput").ap()
    w1T_d = nc.dram_tensor("w1T", [D, H], F32R, kind="ExternalInput").ap()
    w3T_d = nc.dram_tensor("w3T", [D, H], F32R, kind="ExternalInput").ap()
    w2T_d = nc.dram_tensor("w2T", [H, D], F32R, kind="ExternalInput").ap()
    scl_d = nc.dram_tensor("scale_b", [P, C], F32, kind="ExternalInput").ap()
    yT_d = nc.dram_tensor("yT", [D, C], F32, kind="ExternalOutput").ap()

    # DRAM views with the 128-partition k-tile split exposed
    xgT_v = xgT_d.rearrange("(k p) c -> p k c", p=P)     # [P, KD, C]
    w1T_v = w1T_d.rearrange("(k p) h -> p k h", p=P)     # [P, KD, H]
    w3T_v = w3T_d.rearrange("(k p) h -> p k h", p=P)
    w2T_v = w2T_d.rearrange("(k p) d -> p k d", p=P)     # [P, KH, D]

    with tile.TileContext(nc) as tc:
        with tc.tile_pool(name="resident", bufs=1) as res_pool, \
             tc.tile_pool(name="w13", bufs=3) as w13_pool, \
             tc.tile_pool(name="w2", bufs=3) as w2_pool, \
             tc.tile_pool(name="ev", bufs=3) as ev_pool, \
             tc.tile_pool(name="psum", bufs=2, space="PSUM") as ps_pool:

            # xg loaded in (token-chunk, k) slices on the sync (HWDGE) queue
            # so stage A's first psum groups only gate on their own slice,
            # while the w1/w3 stream runs in parallel on the gpsimd queue
            # (one tile per queue: cross-queue writes into a single tile
            # break the DMA->matmul ordering).
            # One 3D DMA per token chunk: chunk granularity is what the
            # matmul deps need (a psum group consumes all 8 k-slices), and
            # fewer dma_starts cut per-transfer overhead on the head stream.
            xg = res_pool.tile([P, KD, C], F32R, tag="xg")
            for t in range(nt):
                nc.sync.dma_start(xg[:, :, tsls[t]], xgT_v[:, :, tsls[t]])
            act = res_pool.tile([P, KH, C], F32R, tag="act")

            # ---- stage A: act[H, C] = silu(w1 @ xgT) * (w3 @ xgT) ----
            # h-blocks processed in pairs with the token-chunk loop outside
            # the pair: two h-blocks of chunk-t compute run before chunk t+1
            # is touched, hiding the next xg chunk's DMA arrival.
            for hp in range(0, HB, 2):
                pair = [h for h in (hp, hp + 1) if h < HB]
                w1ts, w3ts = [], []
                for i, h in enumerate(pair):
                    w1t = w13_pool.tile([P, KD, P], F32R, tag=f"w1_{i}",
                                        bufs=2, name=f"w1t_{h}")
                    nc.gpsimd.dma_start(w1t[:], w1T_v[:, :, h * P:(h + 1) * P])
                    w3t = w13_pool.tile([P, KD, P], F32R, tag=f"w3_{i}",
                                        bufs=2, name=f"w3t_{h}")
                    nc.gpsimd.dma_start(w3t[:], w3T_v[:, :, h * P:(h + 1) * P])
                    w1ts.append(w1t)
                    w3ts.append(w3t)
                for t in range(nt):
                    tsl = tsls[t]
                    for i, h in enumerate(pair):
                        ph1 = ps_pool.tile([P, chunks[t]], F32, tag="h1",
                                           bufs=3, name=f"ph1_{h}_{t}")
                        for k in range(KD):
                            nc.tensor.matmul(ph1[:], w1ts[i][:, k, :],
                                             xg[:, k, tsl],
                                             start=(k == 0), stop=(k == KD - 1))
                        ph3 = ps_pool.tile([P, chunks[t]], F32, tag="h3",
                                           bufs=3, name=f"ph3_{h}_{t}")
                        for k in range(KD):
                            nc.tensor.matmul(ph3[:], w3ts[i][:, k, :],
                                             xg[:, k, tsl],
                                             start=(k == 0), stop=(k == KD - 1))
                        asl = act[:, h, tsl]
                        nc.scalar.activation(asl, ph1[:], func=_ACT_FUNC)
                        nc.vector.tensor_mul(asl, asl, ph3[:])

            # combine-weight row (needed only for stage B evictions)
            scl = res_pool.tile([P, C], F32, tag="scl")
            nc.gpsimd.dma_start(scl[:], scl_d[:, :])

            # ---- stage B: yT[D, C] = (w2 @ act) * scale ----
            for d in range(DB):
                w2t = w2_pool.tile([P, KH, P], F32R, tag="w2")
                nc.sync.dma_start(w2t[:], w2T_v[:, :, d * P:(d + 1) * P])
                for t in range(nt):
                    tsl = tsls[t]
                    py = ps_pool.tile([P, chunks[t]], F32, tag="y")
                    for k in range(KH):
                        nc.tensor.matmul(py[:], w2t[:, k, :], act[:, k, tsl],
                                         start=(k == 0), stop=(k == KH - 1))
                    ysb = ev_pool.tile([P, chunks[t]], F32, tag="ysb")
                    nc.vector.tensor_mul(ysb[:], py[:], scl[:, tsl])
                    nc.scalar.dma_start(yT_d[d * P:(d + 1) * P, tsl], ysb[:])

    nc.compile()
    return nc


def _route(flat, gate_w):
    """Host replica of the reference router. Returns top-2 expert ids and
    combine weights (top-2 of softmax, renormalized)."""
    logits = flat @ gate_w.T                                   # [N, E] f32
    m = logits.max(axis=1, keepdims=True)
    p = np.exp((logits - m).astype(np.float32))
    probs = p / p.sum(axis=1, keepdims=True)
    idx = np.argsort(-probs, axis=1, kind="stable")[:, :TOPK]  # [N, 2]
    top = np.take_along_axis(probs, idx, axis=1)               # [N, 2]
    wn = top / top.sum(axis=1, keepdims=True)
    return idx, wn


def kernel(x, gate_w, w1, w2, w3):
    global LAST_RESULTS
    x = np.asarray(x, np.float32)
    gate_w = np.asarray(gate_w, np.float32)
    w1 = np.asarray(w1, np.float32)
    w2 = np.asarray(w2, np.float32)
    w3 = np.asarray(w3, np.float32)

    flat = x.reshape(N, D)
    idx, wn = _route(flat, gate_w)

    sels, wsels = [], []
    for e in range(E):
        hit = idx == e                                         # [N, 2]
        sel = np.nonzero(hit.any(axis=1))[0]
        k = hit[sel, 1].astype(np.int64)                       # which top slot
        sels.append(sel)
        wsels.append(wn[sel, k])
    cmax = max(len(s) for s in sels)
    chunks = _chunk_plan(cmax)
    C = sum(chunks)

    xT = np.ascontiguousarray(flat.T)                          # [D, N]
    in_maps = []
    for e in range(E):
        sel = sels[e][:C]                  # tokens beyond C go to _host_ffn
        xgT = np.zeros((D, C), np.float32)
        xgT[:, :len(sel)] = xT[:, sel]
        scale_b = np.zeros((P, C), np.float32)
        scale_b[:, :len(sel)] = wsels[e][:C][None, :]
        in_maps.append({
            "xgT": xgT,
            "w1T": np.ascontiguousarray(w1[e].T),
            "w3T": np.ascontiguousarray(w3[e].T),
            "w2T": np.ascontiguousarray(w2[e].T),
            "scale_b": scale_b,
        })

    key = tuple(chunks)
    if key not in _program_cache:
        _program_cache[key] = _build_program(chunks)
    nc = _program_cache[key]

    res = run_bass_kernel_spmd(
        nc, in_maps, core_ids=list(range(E)),
        trace=TRACE,
        trace_cores=list(range(E)) if (TRACE and TRACE_ALL_CORES) else None,
    )
    LAST_RESULTS = res

    out = np.zeros((N, D), np.float32)
    for e in range(E):
        sel = sels[e][:C]
        out[sel] += res.results[e]["yT"][:, :len(sel)].T
        over = sels[e][C:]
        if len(over):
            out[over] += _host_ffn(flat[over], w1[e], w2[e], w3[e],
                                   wsels[e][C:])
    return out.reshape(B, T, D)



# revision 3
# speedup vs baseline: 1.1631x; 1.1631x over previous
"""MoE (top-2 of 8 experts, SwiGLU) Trainium2 kernel, expert-parallel over 8 cores.

Contract: kernel(**inputs) takes the FULL unsharded inputs
  x [2,2048,1024] f32, gate_w [8,1024] f32,
  w1 [8,2048,1024] f32, w2 [8,1024,2048] f32, w3 [8,2048,1024] f32
and returns the FULL output [2,2048,1024] f32.

Strategy (expert-parallel): routing (gate softmax + top-2) runs on host;
tokens are gathered per expert; core e runs the SwiGLU FFN of expert e over
its assigned tokens padded to capacity C (= N*TOPK/E rounded to chunks);
capacity-overflow tokens (<=64/expert) take the host FFN; the host
scatter-adds the two expert contributions per token.

Device kernel (per core), all matmul operands fp16 (same 1 cycle/row PE rate
as fp32r at these sizes, half the DMA/SBUF traffic, lower PE power):
  h1T = w1 @ xgT   [H, C]
  h3T = w3 @ xgT   [H, C]
  aT  = silu(h1T) * h3T      (ACT Silu psum->sbuf fp16, DVE mul)
  yT  = (w2 @ aT) * combine  [D, C]  (DVE mul on psum eviction, fp16 out)

All DRAM tensors use partition-major tile layouts so every DMA moves
2-32KB contiguous runs per partition (the packet rate, not bandwidth, was
the v1 bottleneck).  xg is DMAed in per-k slices so the first matmul only
gates on ~256KB; w1+w3 are fetched as one 512KB tile per h-block on the
two hardware DGE queues (sync/scalar) plus gpsimd for the late blocks;
all of w2 is prefetched during stage A as a single resident tile.
A short burst of dummy matmuls at t=0 ramps the PE clock (0.65->2.4GHz
p-state) while the first DMAs are in flight.
"""

import math
import sys

import numpy as np

for _p in ("/opt/trn_rl_repo", "/opt/pypackages"):
    if _p not in sys.path:
        sys.path.append(_p)

import concourse.bass as bass  # noqa: E402
import concourse.tile as tile  # noqa: E402
from concourse import bacc, mybir  # noqa: E402
from concourse.bass_utils import run_bass_kernel_spmd  # noqa: E402

B, T, D, H, E, TOPK = 2, 2048, 1024, 2048, 8, 2
N = B * T
P = 128
KD = D // P   # 8  k-tiles over D
KH = H // P   # 16 k-tiles over H
HB = H // P   # 16 h blocks of 128 (M dim, stage A)
DB = D // P   # 8  d blocks of 128 (M dim, stage B)

F32 = mybir.dt.float32
F16 = mybir.dt.float16

# set by test.py to capture an NTFF profile; kernel() stores results here
TRACE = False
TRACE_ALL_CORES = False
LAST_RESULTS = None

_program_cache = {}

# CoreSim doesn't implement Silu; simcheck can override this to Sigmoid.
_ACT_FUNC = mybir.ActivationFunctionType.Silu

# dummy matmuls at t=0 to ramp the PE p-state while DMAs land
WARMUP_MM = 12

# Max tokens per expert handled on host when the count barely exceeds a
# 512 multiple (capacity-factor overflow).
OVERFLOW_MAX = 64


def _chunk_plan(cmax: int) -> list[int]:
    """Token-chunk sizes for the device capacity: each <=512 (PSUM bank),
    as equal as possible, 32-aligned, minimal total padding. If cmax is
    within OVERFLOW_MAX above a 512 multiple, use full 512 chunks and let
    the caller route the overflow tokens to the host FFN."""
    if cmax >= 512 and cmax - (cmax // 512) * 512 <= OVERFLOW_MAX:
        return [512] * (cmax // 512)
    n = max(1, math.ceil(cmax / 512))
    chunks = []
    rem = cmax
    for i in range(n):
        s = math.ceil(rem / (n - i) / 32) * 32
        s = min(max(s, 256), 512)
        chunks.append(s)
        rem -= s
    return chunks


def _host_ffn(x_rows, w1e, w2e, w3e, wts):
    """Exact host-side SwiGLU FFN for capacity-overflow tokens (<=64/expert)."""
    h1 = x_rows @ w1e.T
    h3 = x_rows @ w3e.T
    a = h1 / (1.0 + np.exp(-h1)) * h3
    return (a @ w2e.T) * wts[:, None]


def _build_program(chunks: list[int]):
    """Bass program for one core: expert FFN over C = sum(chunks) tokens."""
    C = sum(chunks)
    offs = [sum(chunks[:i]) for i in range(len(chunks))]
    tsls = [bass.ds(o, s) for o, s in zip(offs, chunks)]
    nt = len(chunks)

    nc = bacc.Bacc(
        "TRN2", target_bir_lowering=False, debug=False,
        enable_asserts=False, num_devices=8,
    )
    # partition-major layouts: per-partition runs are contiguous in DRAM
    xg_d = nc.dram_tensor("xg", [P, KD, C], F16, kind="ExternalInput").ap()
    w13_d = nc.dram_tensor("w13", [HB, P, 2, KD, P], F16,
                           kind="ExternalInput").ap()
    w2_d = nc.dram_tensor("w2a", [P, DB, KH, P], F16,
                          kind="ExternalInput").ap()
    scl_d = nc.dram_tensor("scale_b", [P, C], F32, kind="ExternalInput").ap()
    yT_d = nc.dram_tensor("yT", [DB, P, C], F16, kind="ExternalOutput").ap()

    # which DMA queue fetches each h-block's w1/w3 tile:
    #   sync (hw) warms the head, scalar (hw) the middle, gpsimd (sw,
    #   ~13us spin-up) the tail blocks that aren't needed until t>70us.
    def w13_queue(hb):
        if hb < 6:
            return nc.sync
        if hb < 10:
            return nc.scalar
        return nc.gpsimd

    with tile.TileContext(nc) as tc:
        with tc.tile_pool(name="resident", bufs=1) as res_pool, \
             tc.tile_pool(name="w13", bufs=6) as w13_pool, \
             tc.tile_pool(name="ev", bufs=3) as ev_pool, \
             tc.tile_pool(name="psum", bufs=2, space="PSUM") as ps_pool:

            # PE p-state warmup: dummy matmuls on a zeroed tile while the
            # first DMAs are still in flight.
            warm = res_pool.tile([P, 256], F16, tag="warm")
            nc.vector.memset(warm[:], 0.0)
            pwarm = ps_pool.tile([P, 256], F32, tag="pwarm", bufs=1)
            for _ in range(WARMUP_MM):
                nc.tensor.matmul(pwarm[:], warm[:, 0:P], warm[:],
                                 start=True, stop=True)

            # xg per-k slices (2KB/partition runs) on the sync hw queue:
            # the first psum group gates only on slice k=0.
            xg = res_pool.tile([P, KD, C], F16, tag="xg")
            for k in range(KD):
                nc.sync.dma_start(xg[:, k, :], xg_d[:, k, :])

            act = res_pool.tile([P, KH, C], F16, tag="act")

            # combine-weight row (stage B evictions; scalar hw queue, early)
            scl = res_pool.tile([P, C], F32, tag="scl")
            nc.scalar.dma_start(scl[:], scl_d[:, :])

            # all of w2 as one resident tile (32KB/partition contiguous),
            # fetched on sync behind xg + head w13 blocks; needed ~100us in.
            w2all = res_pool.tile([P, DB, KH, P], F16, tag="w2all")

            # ---- stage A: act[H, C] = silu(w1 @ xgT) * (w3 @ xgT) ----
            w2_issued = False
            for hb in range(HB):
                w13t = w13_pool.tile([P, 2, KD, P], F16, tag="w13",
                                     name=f"w13_{hb}")
                w13_queue(hb).dma_start(w13t[:], w13_d[hb])
                if hb == 5:
                    # enqueue after the head w13 blocks on sync
                    nc.sync.dma_start(w2all[:], w2_d[:])
                    w2_issued = True
                for t in range(nt):
                    tsl = tsls[t]
                    ph1 = ps_pool.tile([P, chunks[t]], F32, tag="h1",
                                       bufs=3, name=f"ph1_{hb}_{t}")
                    for k in range(KD):
                        nc.tensor.matmul(ph1[:], w13t[:, 0, k, :],
                                         xg[:, k, tsl],
                                         start=(k == 0), stop=(k == KD - 1))
                    ph3 = ps_pool.tile([P, chunks[t]], F32, tag="h3",
                                       bufs=2, name=f"ph3_{hb}_{t}")
                    for k in range(KD):
                        nc.tensor.matmul(ph3[:], w13t[:, 1, k, :],
                                         xg[:, k, tsl],
                                         start=(k == 0), stop=(k == KD - 1))
                    asl = act[:, hb, tsl]
                    nc.scalar.activation(asl, ph1[:], func=_ACT_FUNC)
                    nc.vector.tensor_mul(asl, asl, ph3[:])
            if not w2_issued:
                nc.sync.dma_start(w2all[:], w2_d[:])

            # ---- stage B: yT[D, C] = (w2 @ act) * scale ----
            for d in range(DB):
                for t in range(nt):
                    tsl = tsls[t]
                    py = ps_pool.tile([P, chunks[t]], F32, tag="y")
                    for k in range(KH):
                        nc.tensor.matmul(py[:], w2all[:, d, k, :],
                                         act[:, k, tsl],
                                         start=(k == 0), stop=(k == KH - 1))
                    ysb = ev_pool.tile([P, chunks[t]], F16, tag="ysb")
                    nc.vector.tensor_mul(ysb[:], py[:], scl[:, tsl])
                    nc.scalar.dma_start(yT_d[d, :, tsl], ysb[:])

    nc.compile()
    return nc


def _route(flat, gate_w):
    """Host replica of the reference router. Returns top-2 expert ids and
    combine weights (top-2 of softmax, renormalized)."""
    logits = flat @ gate_w.T                                   # [N, E] f32
    m = logits.max(axis=1, keepdims=True)
    p = np.exp((logits - m).astype(np.float32))
    probs = p / p.sum(axis=1, keepdims=True)
    idx = np.argsort(-probs, axis=1, kind="stable")[:, :TOPK]  # [N, 2]
    top = np.take_along_axis(probs, idx, axis=1)               # [N, 2]
    wn = top / top.sum(axis=1, keepdims=True)
    return idx, wn


def kernel(x, gate_w, w1, w2, w3):
    global LAST_RESULTS
    x = np.asarray(x, np.float32)
    gate_w = np.asarray(gate_w, np.float32)
    w1 = np.asarray(w1, np.float32)
    w2 = np.asarray(w2, np.float32)
    w3 = np.asarray(w3, np.float32)

    flat = x.reshape(N, D)
    idx, wn = _route(flat, gate_w)

    sels, wsels = [], []
    for e in range(E):
        hit = idx == e                                         # [N, 2]
        sel = np.nonzero(hit.any(axis=1))[0]
        k = hit[sel, 1].astype(np.int64)                       # which top slot
        sels.append(sel)
        wsels.append(wn[sel, k])
    cmax = max(len(s) for s in sels)
    chunks = _chunk_plan(cmax)
    C = sum(chunks)

    in_maps = []
    for e in range(E):
        sel = sels[e][:C]                  # tokens beyond C go to _host_ffn
        n = len(sel)
        # xg[p, k, c] = x[sel[c], k*128+p]
        xg = np.zeros((P, KD, C), np.float16)
        xs = flat[sel].astype(np.float16)                 # [n, D]
        xg[:, :, :n] = xs.T.reshape(KD, P, n).transpose(1, 0, 2)
        # w13[hb, p, s, k, f] = w{1,3}[e][hb*128+f, k*128+p]
        t1 = w1[e].astype(np.float16).reshape(HB, P, KD, P).transpose(0, 3, 2, 1)
        t3 = w3[e].astype(np.float16).reshape(HB, P, KD, P).transpose(0, 3, 2, 1)
        w13 = np.ascontiguousarray(
            np.stack([t1, t3], axis=2))                   # [HB, P, 2, KD, P]
        # w2a[p, db, k, f] = w2[e][db*128+f, k*128+p]
        w2a = np.ascontiguousarray(
            w2[e].astype(np.float16).reshape(DB, P, KH, P)
            .transpose(3, 0, 2, 1))                       # [P, DB, KH, P]
        scale_b = np.zeros((P, C), np.float32)
        scale_b[:, :n] = wsels[e][:C][None, :]
        in_maps.append({
            "xg": xg,
            "w13": w13,
            "w2a": w2a,
            "scale_b": scale_b,
        })

    key = tuple(chunks)
    if key not in _program_cache:
        _program_cache[key] = _build_program(chunks)
    nc = _program_cache[key]

    res = run_bass_kernel_spmd(
        nc, in_maps, core_ids=list(range(E)),
        trace=TRACE,
        trace_cores=list(range(E)) if (TRACE and TRACE_ALL_CORES) else None,
    )
    LAST_RESULTS = res

    out = np.zeros((N, D), np.float32)
    for e in range(E):
        sel = sels[e][:C]
        # yT[db, p, c] -> [c, d]
        y = res.results[e]["yT"].astype(np.float32)
        y = y.transpose(2, 0, 1).reshape(C, D)
        out[sel] += y[:len(sel)]
        over = sels[e][C:]
        if len(over):
            out[over] += _host_ffn(flat[over], w1[e], w2[e], w3[e],
                                   wsels[e][C:])
    return out.reshape(B, T, D)
